# revision 40
# baseline (speedup 1.0000x reference)
"""Banded dense-dilated KNN graph (k=9, band 90, dilation 1) on 8 Trainium2 cores.

Input  x: (4, 64, 8192, 1) float32.
Output e: (2, 4, 8192, 9) int32 = stack([nn_idx, center_idx]).

Algorithm (packed single-pass top-8)
------------------------------------
Per row i the reference takes the 9 smallest banded distances over
j in [i-89, i], which (after L2 normalization) is the ordering of the dot
products u_i.u_j descending, with rank 0 always the self column.  The host
normalizes x and ships bf16 u; the device computes, per pair of 128-row
blocks, the two [128 x 216] windows of dots with bf16 PE matmuls accumulated
on top of a mask plane (0 valid / -2^30 invalid) placed by a K=128 identity
matmul, so no elementwise masking pass exists.  The ACT engine then copies
the fp32 PSUM pair to bf16 written at stride 2 into the HIGH halves of a
uint32 SBUF tile whose LOW halves were pre-filled once (gpsimd iota) with
the column index: each 4-byte slot becomes the fp32 number
(bf16(dot) << 16 | col), whose ordering equals (dot, col) ordering.  A
single DVE max8 pass per block then yields the top-8 values AND indices (in
the low 16 bits); no max_index pass is needed.  The host extracts indices
from the value bits, rebuilds self/rank-0, the first-8-row head fixup, and
the center plane (all exact).

Quantizing the dots to bf16 perturbs only near-ties, which are band-local
index swaps; measured end-to-end rel err ~9e-4 (gate 2e-2).

Sharding: 8 cores = 4 batches x 2 row-halves of 4096 rows; no cross-core
communication.  On-chip the 4185 columns are stacked into a [128 x 2137]
layout (two 64-partition halves overlapping by 89 columns).

Schedule notes (cost-model driven):
- identity+first-pair mask ride the first HWDGE DMA; the remaining mask
  plane takes the parallel Pool-SWDGE path; input streams in 3 chunks.
- dummy PE matmuls ride out the input fill so the tensor engine never
  blocks on a semaphore (a blocking matmul resets the p-state ramp and
  the whole queued matmul burst would be costed at the slow p-state).
- the DVE max8 stream (32 x 285 ns) is the bottleneck and runs with zero
  bubbles; output stores are split 4 ways with a tiny final store so the
  post-stream tail is one short DMA chain.
"""

import sys

import numpy as np

for _p in ("/opt/trn_rl_repo", "/root/.axon_site/_ro/trn_rl_repo"):
    if _p not in sys.path:
        sys.path.append(_p)

B = 4
D = 64
N = 8192
K = 9
LB = 90  # band width (j in [i-89, i])
W = LB - 1  # 89 back-columns
HALF = N // 2  # rows per core
NCOLS = W + HALF  # 4185 input columns per core
NBLK = HALF // 128  # 32 row blocks per core
HALF_BLK = NBLK // 2  # 16 blocks per stacked half
HCOLS = W + HALF_BLK * 128  # 2137 columns per stacked half
WIN = 128 + W - 1  # 216-column window (cols r..r+88 for r in [0,127])
PAIRW = 2 * WIN  # 432: two blocks share one PSUM bank + one ACT copy
NP = 4  # packed-tile / psum double-buffering depth
BIG = 2.0**30
NWARM = 10  # dummy PE matmuls riding out the input-DMA fill
# blocks covered by each output store (last kept tiny to shrink the tail)
STORE_EDGES = [0, 12, 20, 30, NBLK]
# 16 block pairs; each pair shares one PSUM bank, one ACT copy, one mask mm
GROUPS = [(t, t + 1) for t in range(0, NBLK, 2)]

_CACHED = {}

# Input stream chunks (columns of the stacked [128, 2137] slab); the first
# chunk covers the first two pairs so the DVE stream starts bubble-free.
CHUNKS = [(0, 601), (601, 512), (1113, 1024)]


def _build_masks():
    # mask[r, c] = 0 where window column c is a valid neighbor of block row r,
    # -2^30 otherwise.  Valid (non-self) neighbors of global row i = r0 + r
    # are j in [i-89, i-1] -> c = j - (r0 - 89) in [r, r+88].
    r = np.arange(128)[:, None]
    c = np.arange(WIN)[None, :]
    valid = (c >= r) & (c <= r + W - 1)
    m_rest = np.where(valid, 0.0, -BIG).astype(np.float32)
    # Block 0 of a batch-leading half additionally requires j >= 0 (c >= 89).
    m_first = np.where(valid & (c >= W), 0.0, -BIG).astype(np.float32)
    return m_first, m_rest


def _build_bass():
    import concourse.mybir as mybir
    from concourse import bacc
    from concourse.tile import TileContext

    f32 = mybir.dt.float32
    bf16 = mybir.dt.bfloat16
    u32 = mybir.dt.uint32
    Act = mybir.ActivationFunctionType

    nc = bacc.Bacc("TRN2", target_bir_lowering=False, debug=False, num_devices=8)
    # cna: [0:128] identity, [128:560] first pair's mask plane (first | rest)
    # cnb: [0:432] mask plane for all other pairs (rest | rest)
    cna_d = nc.dram_tensor("cna", [128, 128 + PAIRW], bf16, kind="ExternalInput")
    cnb_d = nc.dram_tensor("cnb", [128, PAIRW], bf16, kind="ExternalInput")
    xs_d = nc.dram_tensor("xs", [128, HCOLS], bf16, kind="ExternalInput")
    vals_d = nc.dram_tensor("vals", [HALF, 8], f32, kind="ExternalOutput")

    with TileContext(nc) as tc:
        with (
            tc.tile_pool(name="consts", bufs=1) as consts,
            tc.tile_pool(name="pss", bufs=NP, space="PSUM") as pss,
            tc.tile_pool(name="psw", bufs=1, space="PSUM") as psw,
        ):
            # First pair's constants ride HWDGE first (smallest latency to
            # the first matmul); the remaining mask plane takes the parallel
            # Pool SWDGE path.
            CNA = consts.tile([128, 128 + PAIRW], bf16, tag="CNA")
            nc.sync.dma_start(CNA[:], cna_d[:])
            IDT = CNA[:, 0:128]
            X = consts.tile([128, HCOLS], bf16, tag="X")
            for c0, cw in CHUNKS:
                nc.sync.dma_start(X[:, c0 : c0 + cw], xs_d[:, c0 : c0 + cw])
            CNB = consts.tile([128, PAIRW], bf16, tag="CNB")
            nc.gpsimd.dma_start(CNB[:], cnb_d[:])
            VAL = consts.tile([128, NBLK * 8], f32, tag="VAL")

            # Warm the ACT Copy function table, and keep the PE p-state ramp
            # alive with dummy matmuls while the input DMAs are in flight
            # (the dummies must outlast the constants' arrival so PE never
            # blocks on a semaphore, which would reset the p-state ramp).
            wb = consts.tile([2, 256], bf16, tag="wb")
            nc.vector.memset(wb[:], 1.0)
            warm = consts.tile([2, 16], f32, tag="warm")
            nc.vector.memset(warm[:], 1.0)
            nc.scalar.activation(warm[:], warm[:], Act.Copy)
            wp = psw.tile([2, 256], f32, tag="wp")
            for _ in range(NWARM):
                nc.tensor.matmul(
                    wp[:], lhsT=wb[:, 0:2], rhs=wb[:], start=True, stop=True
                )

            # Packed tiles: low uint16 halves hold the column iota (written
            # once); ACT rewrites only the high halves each reuse.
            P = []
            for i in range(NP):
                t = consts.tile([128, PAIRW], f32, tag=f"P{i}")
                nc.gpsimd.iota(
                    t[:].bitcast(u32),
                    pattern=[[0, 2], [1, WIN]],
                    base=0,
                    channel_multiplier=0,
                )
                P.append(t)

            vals_rtk = vals_d.ap().rearrange("(t r) k -> r t k", t=NBLK, r=128)

            for q, grp in enumerate(GROUPS):
                gw = WIN * len(grp)
                pd = pss.tile([128, PAIRW], f32, tag="pd")
                # One mask matmul seeds both windows of the pair; the two
                # dots matmuls then accumulate on top.
                mrhs = (
                    CNA[:, 128 : 128 + gw] if grp[0] == 0 else CNB[:, 0:gw]
                )
                nc.tensor.matmul(
                    pd[:, 0:gw],
                    lhsT=IDT,
                    rhs=mrhs,
                    start=True,
                    stop=False,
                    skip_group_check=True,
                )
                for s, t in enumerate(grp):
                    h, tl = t // HALF_BLK, t % HALF_BLK
                    p0 = 64 * h
                    osl = slice(WIN * s, WIN * (s + 1))
                    nc.tensor.matmul(
                        pd[:, osl],
                        lhsT=X[p0 : p0 + 64, W + 128 * tl : W + 128 * tl + 128],
                        rhs=X[p0 : p0 + 64, 128 * tl : 128 * tl + WIN],
                        start=False,
                        stop=True,
                        skip_group_check=True,
                    )
                pt = P[q % NP]
                hi = pt[:].bitcast(bf16).rearrange(
                    "p (c two) -> p c two", two=2
                )[:, :, 1:2]
                nc.scalar.activation(hi, pd[:], Act.Copy)
                for s, t in enumerate(grp):
                    nc.vector.max(
                        out=VAL[:, 8 * t : 8 * t + 8],
                        in_=pt[:, WIN * s : WIN * (s + 1)],
                    )
                for g0, g1 in zip(STORE_EDGES, STORE_EDGES[1:]):
                    if grp[-1] + 1 == g1:
                        nc.sync.dma_start(
                            vals_rtk[:, g0:g1, :], VAL[:, 8 * g0 : 8 * g1]
                        )

    nc.finalize()
    return nc


LAST_EXEC_NS = None


def kernel(x: np.ndarray) -> np.ndarray:
    global LAST_EXEC_NS
    import os

    import ml_dtypes
    from concourse import bass_utils

    if "nc" not in _CACHED:
        _CACHED["nc"] = _build_bass()
        _CACHED["masks"] = _build_masks()
    nc = _CACHED["nc"]
    m_first, m_rest = _CACHED["masks"]

    x = np.asarray(x)
    assert x.shape == (B, D, N, 1) and x.dtype == np.float32
    xm = x[:, :, :, 0]  # (B, D, N)

    # Host-side L2 normalization over the feature axis (0.2% of the FLOPs);
    # row-side scaling cancels within each row's ranking, but normalizing
    # both sides keeps values in [-1, 1] for bf16.
    norm = np.sqrt(np.sum(xm * xm, axis=1, keepdims=True))
    u = (xm / np.maximum(norm, 1e-12)).astype(ml_dtypes.bfloat16)

    ident = np.eye(128, dtype=np.float32)
    cna_lead = np.concatenate([ident, m_first, m_rest], axis=1).astype(
        ml_dtypes.bfloat16
    )
    cna_tail = np.concatenate([ident, m_rest, m_rest], axis=1).astype(
        ml_dtypes.bfloat16
    )
    cnb = np.concatenate([m_rest, m_rest], axis=1).astype(ml_dtypes.bfloat16)

    in_maps = []
    for core in range(8):
        b, h = core // 2, core % 2
        if h == 0:
            xsf = np.concatenate(
                [np.zeros((D, W), ml_dtypes.bfloat16), u[b, :, 0:HALF]], axis=1
            )
        else:
            xsf = np.ascontiguousarray(u[b, :, HALF - W : N])
        # stack into two overlapping 64-partition halves
        xs = np.concatenate(
            [xsf[:, 0:HCOLS], xsf[:, HALF_BLK * 128 : NCOLS]], axis=0
        )
        in_maps.append(
            {"xs": xs, "cna": cna_lead if h == 0 else cna_tail, "cnb": cnb}
        )

    trace = os.environ.get("KNN_TRACE", "0") == "1"
    res = bass_utils.run_bass_kernel_spmd(
        nc, in_maps, core_ids=list(range(8)), trace=trace
    )
    LAST_EXEC_NS = res.exec_time_ns

    # --- host-side unshard + index reconstruction (exact) ---
    nn = np.empty((B, N, K), np.int64)
    rows = np.arange(HALF)
    offs = (rows // 128) * 128 - W  # window base per local row block
    for core in range(8):
        b, h = core // 2, core % 2
        start = h * HALF
        vals = np.ascontiguousarray(res.results[core]["vals"])  # (HALF, 8) f32
        c = (vals.view(np.uint32) & 0xFFFF).astype(np.int64)
        nn[b, start : start + HALF, 1:] = c + (start + offs)[:, None]
    nn[:, :, 0] = np.arange(N)[None, :]
    # Head fixup: row i < 8 has only i valid non-self neighbors; reference
    # fills columns k > i with the self index.
    for i in range(K - 1):
        nn[:, i, i + 1 :] = i
    center = np.broadcast_to(np.arange(N)[None, :, None], (B, N, K))
    return np.stack([nn, center], axis=0).astype(np.int32)


# revision 41
# speedup vs baseline: 1.0049x; 1.0049x over previous
"""Banded dense-dilated KNN graph (k=9, band 90, dilation 1) on 8 Trainium2 cores.

Input  x: (4, 64, 8192, 1) float32.
Output e: (2, 4, 8192, 9) int32 = stack([nn_idx, center_idx]).

Algorithm (packed single-pass top-8)
------------------------------------
Per row i the reference takes the 9 smallest banded distances over
j in [i-89, i], which (after L2 normalization) is the ordering of the dot
products u_i.u_j descending, with rank 0 always the self column.  The host
normalizes x and ships bf16 u; the device computes, per pair of 128-row
blocks, the two [128 x 216] windows of dots with bf16 PE matmuls accumulated
on top of a mask plane (0 valid / -2^30 invalid) placed by a K=128 identity
matmul, so no elementwise masking pass exists.  The ACT engine then copies
the fp32 PSUM pair to bf16 written at stride 2 into the HIGH halves of a
uint32 SBUF tile whose LOW halves were pre-filled once (gpsimd iota) with
the column index: each 4-byte slot becomes the fp32 number
(bf16(dot) << 16 | col), whose ordering equals (dot, col) ordering.  A
single DVE max8 pass per block then yields the top-8 values AND indices (in
the low 16 bits); no max_index pass is needed.  The host extracts indices
from the value bits, rebuilds self/rank-0, the first-8-row head fixup, and
the center plane (all exact).

Quantizing the dots to bf16 perturbs only near-ties, which are band-local
index swaps; measured end-to-end rel err ~9e-4 (gate 2e-2).

Sharding: 8 cores = 4 batches x 2 row-halves of 4096 rows; no cross-core
communication.  On-chip the 4185 columns are stacked into a [128 x 2137]
layout (two 64-partition halves overlapping by 89 columns).

Schedule notes (cost-model driven):
- identity+first-pair mask ride the first HWDGE DMA; the remaining mask
  plane takes the parallel Pool-SWDGE path; input streams in 3 chunks.
- dummy PE matmuls ride out the input fill so the tensor engine never
  blocks on a semaphore (a blocking matmul resets the p-state ramp and
  the whole queued matmul burst would be costed at the slow p-state).
- the DVE max8 stream (32 x 285 ns) is the bottleneck and runs with zero
  bubbles; output stores are split 4 ways with a tiny final store so the
  post-stream tail is one short DMA chain.
"""

import sys

import numpy as np

for _p in ("/opt/trn_rl_repo", "/root/.axon_site/_ro/trn_rl_repo"):
    if _p not in sys.path:
        sys.path.append(_p)

B = 4
D = 64
N = 8192
K = 9
LB = 90  # band width (j in [i-89, i])
W = LB - 1  # 89 back-columns
HALF = N // 2  # rows per core
NCOLS = W + HALF  # 4185 input columns per core
NBLK = HALF // 128  # 32 row blocks per core
HALF_BLK = NBLK // 2  # 16 blocks per stacked half
HCOLS = W + HALF_BLK * 128  # 2137 columns per stacked half
WIN = 128 + W - 1  # 216-column window (cols r..r+88 for r in [0,127])
PAIRW = 2 * WIN  # 432: two blocks share one PSUM bank + one ACT copy
NP = 4  # packed-tile / psum double-buffering depth
BIG = 2.0**30
NWARM = 10  # dummy PE matmuls riding out the input-DMA fill
# blocks covered by each output store (last kept tiny to shrink the tail)
STORE_EDGES = [0, 12, 20, 30, NBLK]
# 16 block pairs; each pair shares one PSUM bank, one ACT copy, one mask mm
GROUPS = [(t, t + 1) for t in range(0, NBLK, 2)]

_CACHED = {}

# Input stream chunks (columns of the stacked [128, 2137] slab); the first
# chunk covers the first two pairs so the DVE stream starts bubble-free.
CHUNKS = [(0, 345), (345, 520), (865, 1272)]


def _build_masks():
    # mask[r, c] = 0 where window column c is a valid neighbor of block row r,
    # -2^30 otherwise.  Valid (non-self) neighbors of global row i = r0 + r
    # are j in [i-89, i-1] -> c = j - (r0 - 89) in [r, r+88].
    r = np.arange(128)[:, None]
    c = np.arange(WIN)[None, :]
    valid = (c >= r) & (c <= r + W - 1)
    m_rest = np.where(valid, 0.0, -BIG).astype(np.float32)
    # Block 0 of a batch-leading half additionally requires j >= 0 (c >= 89).
    m_first = np.where(valid & (c >= W), 0.0, -BIG).astype(np.float32)
    return m_first, m_rest


def _build_bass():
    import concourse.mybir as mybir
    from concourse import bacc
    from concourse.tile import TileContext

    f32 = mybir.dt.float32
    bf16 = mybir.dt.bfloat16
    u32 = mybir.dt.uint32
    Act = mybir.ActivationFunctionType

    nc = bacc.Bacc("TRN2", target_bir_lowering=False, debug=False, num_devices=8)
    # cna: [0:128] identity, [128:560] first pair's mask plane (first | rest)
    # cnb: [0:432] mask plane for all other pairs (rest | rest)
    cna_d = nc.dram_tensor("cna", [128, 128 + PAIRW], bf16, kind="ExternalInput")
    cnb_d = nc.dram_tensor("cnb", [128, PAIRW], bf16, kind="ExternalInput")
    xs_d = nc.dram_tensor("xs", [128, HCOLS], bf16, kind="ExternalInput")
    vals_d = nc.dram_tensor("vals", [HALF, 8], f32, kind="ExternalOutput")

    with TileContext(nc) as tc:
        with (
            tc.tile_pool(name="consts", bufs=1) as consts,
            tc.tile_pool(name="pss", bufs=NP, space="PSUM") as pss,
            tc.tile_pool(name="psw", bufs=1, space="PSUM") as psw,
        ):
            # First pair's constants ride HWDGE first (smallest latency to
            # the first matmul); the remaining mask plane takes the parallel
            # Pool SWDGE path.
            CNA = consts.tile([128, 128 + PAIRW], bf16, tag="CNA")
            nc.sync.dma_start(CNA[:], cna_d[:])
            IDT = CNA[:, 0:128]
            X = consts.tile([128, HCOLS], bf16, tag="X")
            for c0, cw in CHUNKS:
                nc.sync.dma_start(X[:, c0 : c0 + cw], xs_d[:, c0 : c0 + cw])
            CNB = consts.tile([128, PAIRW], bf16, tag="CNB")
            nc.gpsimd.dma_start(CNB[:], cnb_d[:])
            VAL = consts.tile([128, NBLK * 8], f32, tag="VAL")

            # Warm the ACT Copy function table, and keep the PE p-state ramp
            # alive with dummy matmuls while the input DMAs are in flight
            # (the dummies must outlast the constants' arrival so PE never
            # blocks on a semaphore, which would reset the p-state ramp).
            wb = consts.tile([2, 256], bf16, tag="wb")
            nc.vector.memset(wb[:], 1.0)
            warm = consts.tile([2, 16], f32, tag="warm")
            nc.vector.memset(warm[:], 1.0)
            nc.scalar.activation(warm[:], warm[:], Act.Copy)
            wp = psw.tile([2, 256], f32, tag="wp")
            for _ in range(NWARM):
                nc.tensor.matmul(
                    wp[:], lhsT=wb[:, 0:2], rhs=wb[:], start=True, stop=True
                )

            # Packed tiles: low uint16 halves hold the column iota (written
            # once); ACT rewrites only the high halves each reuse.
            P = []
            for i in range(NP):
                t = consts.tile([128, PAIRW], f32, tag=f"P{i}")
                nc.gpsimd.iota(
                    t[:].bitcast(u32),
                    pattern=[[0, 2], [1, WIN]],
                    base=0,
                    channel_multiplier=0,
                )
                P.append(t)

            vals_rtk = vals_d.ap().rearrange("(t r) k -> r t k", t=NBLK, r=128)

            for q, grp in enumerate(GROUPS):
                gw = WIN * len(grp)
                pd = pss.tile([128, PAIRW], f32, tag="pd")
                # One mask matmul seeds both windows of the pair; the two
                # dots matmuls then accumulate on top.
                mrhs = (
                    CNA[:, 128 : 128 + gw] if grp[0] == 0 else CNB[:, 0:gw]
                )
                nc.tensor.matmul(
                    pd[:, 0:gw],
                    lhsT=IDT,
                    rhs=mrhs,
                    start=True,
                    stop=False,
                    skip_group_check=True,
                )
                for s, t in enumerate(grp):
                    h, tl = t // HALF_BLK, t % HALF_BLK
                    p0 = 64 * h
                    osl = slice(WIN * s, WIN * (s + 1))
                    nc.tensor.matmul(
                        pd[:, osl],
                        lhsT=X[p0 : p0 + 64, W + 128 * tl : W + 128 * tl + 128],
                        rhs=X[p0 : p0 + 64, 128 * tl : 128 * tl + WIN],
                        start=False,
                        stop=True,
                        skip_group_check=True,
                    )
                pt = P[q % NP]
                hi = pt[:].bitcast(bf16).rearrange(
                    "p (c two) -> p c two", two=2
                )[:, :, 1:2]
                nc.scalar.activation(hi, pd[:], Act.Copy)
                for s, t in enumerate(grp):
                    nc.vector.max(
                        out=VAL[:, 8 * t : 8 * t + 8],
                        in_=pt[:, WIN * s : WIN * (s + 1)],
                    )
                for g0, g1 in zip(STORE_EDGES, STORE_EDGES[1:]):
                    if grp[-1] + 1 == g1:
                        nc.sync.dma_start(
                            vals_rtk[:, g0:g1, :], VAL[:, 8 * g0 : 8 * g1]
                        )

    nc.finalize()
    return nc


LAST_EXEC_NS = None


def kernel(x: np.ndarray) -> np.ndarray:
    global LAST_EXEC_NS
    import os

    import ml_dtypes
    from concourse import bass_utils

    if "nc" not in _CACHED:
        _CACHED["nc"] = _build_bass()
        _CACHED["masks"] = _build_masks()
    nc = _CACHED["nc"]
    m_first, m_rest = _CACHED["masks"]

    x = np.asarray(x)
    assert x.shape == (B, D, N, 1) and x.dtype == np.float32
    xm = x[:, :, :, 0]  # (B, D, N)

    # Host-side L2 normalization over the feature axis (0.2% of the FLOPs);
    # row-side scaling cancels within each row's ranking, but normalizing
    # both sides keeps values in [-1, 1] for bf16.
    norm = np.sqrt(np.sum(xm * xm, axis=1, keepdims=True))
    u = (xm / np.maximum(norm, 1e-12)).astype(ml_dtypes.bfloat16)

    ident = np.eye(128, dtype=np.float32)
    cna_lead = np.concatenate([ident, m_first, m_rest], axis=1).astype(
        ml_dtypes.bfloat16
    )
    cna_tail = np.concatenate([ident, m_rest, m_rest], axis=1).astype(
        ml_dtypes.bfloat16
    )
    cnb = np.concatenate([m_rest, m_rest], axis=1).astype(ml_dtypes.bfloat16)

    in_maps = []
    for core in range(8):
        b, h = core // 2, core % 2
        if h == 0:
            xsf = np.concatenate(
                [np.zeros((D, W), ml_dtypes.bfloat16), u[b, :, 0:HALF]], axis=1
            )
        else:
            xsf = np.ascontiguousarray(u[b, :, HALF - W : N])
        # stack into two overlapping 64-partition halves
        xs = np.concatenate(
            [xsf[:, 0:HCOLS], xsf[:, HALF_BLK * 128 : NCOLS]], axis=0
        )
        in_maps.append(
            {"xs": xs, "cna": cna_lead if h == 0 else cna_tail, "cnb": cnb}
        )

    trace = os.environ.get("KNN_TRACE", "0") == "1"
    res = bass_utils.run_bass_kernel_spmd(
        nc, in_maps, core_ids=list(range(8)), trace=trace
    )
    LAST_EXEC_NS = res.exec_time_ns

    # --- host-side unshard + index reconstruction (exact) ---
    nn = np.empty((B, N, K), np.int64)
    rows = np.arange(HALF)
    offs = (rows // 128) * 128 - W  # window base per local row block
    for core in range(8):
        b, h = core // 2, core % 2
        start = h * HALF
        vals = np.ascontiguousarray(res.results[core]["vals"])  # (HALF, 8) f32
        c = (vals.view(np.uint32) & 0xFFFF).astype(np.int64)
        nn[b, start : start + HALF, 1:] = c + (start + offs)[:, None]
    nn[:, :, 0] = np.arange(N)[None, :]
    # Head fixup: row i < 8 has only i valid non-self neighbors; reference
    # fills columns k > i with the self index.
    for i in range(K - 1):
        nn[:, i, i + 1 :] = i
    center = np.broadcast_to(np.arange(N)[None, :, None], (B, N, K))
    return np.stack([nn, center], axis=0).astype(np.int32)
